# revision 1
# baseline (speedup 1.0000x reference)
"""Causal self-attention (B=2, S=4096, D=512, H=8) on 8 Trainium2 cores.

Sharding: core c handles batch b = c//4 and heads {2*(c%4), 2*(c%4)+1}.
Each core computes q/k/v projections for its two heads, causal flash-style
attention in a transposed (k-major) score layout, and per-head undivided
output-projection partials po_h^T = Wo_h @ attn_h^T plus the softmax
denominators.  The host divides by the denominators, sums the 4 cores per
batch, adds bo, and transposes back.

Device layout notes:
  qT/kT: [128, S] bf16, rows 0-63 head0, 64-127 head1 (head dim on
  partitions).  scores^T tiles: [128 keys, 1024 queries]; exp on ACT reads
  PSUM directly with the padding mask folded into the per-partition bias
  and 1/sqrt(hd) into the scale, writing bf16.  V is PE-transposed to
  k-major [128, 65]-blocks with a ones column appended, so the PV matmul
  accumulates numerators and the softmax denominator (row 64) together.
  All matmul operands are bf16 (full-rate PE + fast weight load);
  accumulation stays fp32 in PSUM.
"""

import sys

sys.path.insert(0, "/opt/trn_rl_repo")

from contextlib import ExitStack

import ml_dtypes
import numpy as np

import concourse.bass as bass
import concourse.tile as tile
from concourse import bacc, bass_utils, mybir

B, S, D = 2, 4096, 512
H, HD = 8, 64
NCORES = 8
F32 = mybir.dt.float32
BF16 = mybir.dt.bfloat16
EXP = mybir.ActivationFunctionType.Exp
NPBF16 = ml_dtypes.bfloat16

CHUNK = 1024                  # query-chunk width
NCHUNK = S // CHUNK           # 4
KBLK = 128                    # key block (partition dim)
KB_PER_CHUNK = CHUNK // KBLK  # 8
NEG = -1.0e30


def _pieces(col0):
    """Split [col0, CHUNK) into <=512-wide pieces aligned to 512 boundaries."""
    out = []
    c = col0
    while c < CHUNK:
        nxt = min(CHUNK, (c // 512 + 1) * 512)
        out.append((c, nxt))
        c = nxt
    return out


def _emit(nc, tc, ctx, io):
    xT, wq_p, wk_p, wv_p, wo01d, bqkv, kbias, trimask, ident2, \
        po0T, po1T, dens = io

    const = ctx.enter_context(tc.tile_pool(name="const", bufs=1))
    poolA = ctx.enter_context(tc.tile_pool(name="poolA", bufs=1))
    poolB = ctx.enter_context(tc.tile_pool(name="poolB", bufs=1))

    # ---- constants / weights into SBUF ----
    wq_sb = const.tile([128, 512], BF16, tag="wq")
    wk_sb = const.tile([128, 512], BF16, tag="wk")
    wv_sb = const.tile([128, 512], BF16, tag="wv")
    wo01_sb = const.tile([128, 512], BF16, tag="wo01")
    bqkv_sb = const.tile([128, 3], F32, tag="bqkv")
    kbias_sb = const.tile([128, 32], F32, tag="kbias")
    tri_sb = const.tile([128, 128], BF16, tag="tri")
    id2_sb = const.tile([128, 64], BF16, tag="id2")
    onesf_sb = const.tile([128, 1], F32, tag="onesf")
    nc.vector.memset(onesf_sb[:], 1.0)
    for t, a in ((wq_sb, wq_p), (wk_sb, wk_p), (wv_sb, wv_p),
                 (bqkv_sb, bqkv), (id2_sb, ident2), (kbias_sb, kbias),
                 (tri_sb, trimask), (wo01_sb, wo01d)):
        nc.sync.dma_start(t[:], a[:])

    # ---- intermediates: poolA spans phases 1-2, poolB phases 2-3 ----
    qT = poolA.tile([128, S], BF16, tag="qT")
    kT = poolA.tile([128, S], BF16, tag="kT")
    v0 = poolA.tile([128, 32 * 65], BF16, tag="v0")
    v1 = poolA.tile([128, 32 * 65], BF16, tag="v1")
    oT01 = poolB.tile([128, S], BF16, tag="oT01")
    den0 = poolB.tile([1, S], F32, tag="den0")
    den1 = poolB.tile([1, S], F32, tag="den1")

    # ---- phase 1: q/k/v projections (+ v transpose to k-major) ----
    with tc.tile_pool(name="ph1sb", bufs=1) as p1s, \
         tc.tile_pool(name="ph1ps", bufs=2, space="PSUM") as p1p:
        vT = p1s.tile([128, S], BF16, tag="vT")

        for J in range(NCHUNK):
            x_sb = []
            for ks in range(4):
                xt = p1s.tile([128, CHUNK], BF16, tag=f"x{ks}", bufs=2)
                nc.sync.dma_start(
                    xt[:],
                    xT[ks * 128:(ks + 1) * 128, J * CHUNK:(J + 1) * CHUNK])
                x_sb.append(xt)
            for w_sb, bcol, dest in ((wq_sb, 0, qT), (wk_sb, 1, kT),
                                     (wv_sb, 2, vT)):
                ps = p1p.tile([128, CHUNK], F32, tag="proj")
                for half in range(2):
                    lo = half * 512
                    for ks in range(4):
                        nc.tensor.matmul(
                            ps[:, half * 512:(half + 1) * 512],
                            w_sb[:, ks * 128:(ks + 1) * 128],
                            x_sb[ks][:, lo:lo + 512],
                            start=(ks == 0), stop=(ks == 3),
                        )
                nc.vector.tensor_scalar_add(
                    dest[:, J * CHUNK:(J + 1) * CHUNK], ps[:],
                    bqkv_sb[:, bcol:bcol + 1])

        # V -> k-major blocks; head0/head1 transposes issued adjacently so
        # they run concurrently on PE row-groups 0-63 / 64-127
        for g in range(4):  # groups of 8 key-blocks
            tr0 = p1p.tile([128, 512], BF16, tag="vtr0")
            tr1 = p1p.tile([128, 512], BF16, tag="vtr1")
            for i in range(8):
                kb = g * 8 + i
                for hh, tr in ((0, tr0), (1, tr1)):
                    nc.tensor.transpose(
                        tr[:, i * 64:(i + 1) * 64],
                        vT[hh * 64:(hh + 1) * 64, kb * KBLK:(kb + 1) * KBLK],
                        id2_sb[hh * 64:(hh + 1) * 64, :],
                    )
            for tr, vdst in ((tr0, v0), (tr1, v1)):
                dst = vdst[:, g * 8 * 65:(g + 1) * 8 * 65]
                dst = dst.rearrange("p (k c) -> p k c", c=65)[:, :, 0:64]
                nc.vector.tensor_copy(
                    dst, tr.rearrange("p (k c) -> p k c", c=64))
        for vdst in (v0, v1):
            ones_col = vdst.rearrange("p (k c) -> p k c", c=65)[:, :, 64:65]
            nc.vector.tensor_copy(
                ones_col, onesf_sb[:].to_broadcast((128, 32, 1)))

    # ---- phase 2: attention, heads interleaved so the K=64 QK matmuls of
    # head0/head1 run concurrently on PE row-groups 0-63 / 64-127 ----
    with tc.tile_pool(name="etp", bufs=8) as etp, \
         tc.tile_pool(name="ps_st", bufs=1, space="PSUM") as ps_st, \
         tc.tile_pool(name="ps_pv", bufs=1, space="PSUM") as ps_pv:
        for J in range(NCHUNK):
            pv0 = ps_pv.tile([65, CHUNK], F32, tag="pv0")
            pv1 = ps_pv.tile([65, CHUNK], F32, tag="pv1")
            nkb = KB_PER_CHUNK * (J + 1)
            for kb in range(nkb):
                p = kb - KB_PER_CHUNK * J
                col0 = KBLK * p if p >= 0 else 0
                pieces = _pieces(col0)
                st0 = ps_st.tile([128, CHUNK], F32, tag="st0")
                st1 = ps_st.tile([128, CHUNK], F32, tag="st1")
                for st, hh in ((st0, 0), (st1, 1)):
                    hsl = slice(hh * 64, (hh + 1) * 64)
                    for (a, b) in pieces:
                        nc.tensor.matmul(
                            st[:, a:b],
                            kT[hsl, kb * KBLK:(kb + 1) * KBLK],
                            qT[hsl, J * CHUNK + a:J * CHUNK + b],
                            start=True, stop=True,
                        )
                ets = []
                for st in (st0, st1):
                    et = etp.tile([128, CHUNK], BF16, tag="et")
                    nc.scalar.activation(
                        et[:, col0:], st[:, col0:], EXP,
                        bias=kbias_sb[:, kb:kb + 1], scale=0.125)
                    if p >= 0:
                        nc.vector.tensor_mul(
                            et[:, col0:col0 + KBLK], et[:, col0:col0 + KBLK],
                            tri_sb[:])
                    ets.append(et)
                for et, vsb, pv in ((ets[0], v0, pv0), (ets[1], v1, pv1)):
                    for (a, b) in pieces:
                        # stop exactly on the last matmul touching each
                        # 512-wide psum bank region
                        last_a = (kb == KB_PER_CHUNK * J + 3 and a < 512)
                        last_b = (kb == nkb - 1)
                        nc.tensor.matmul(
                            pv[:, a:b],
                            vsb[:, kb * 65:(kb + 1) * 65],
                            et[:, a:b],
                            start=(kb == 0),
                            stop=(last_a if a < 512 else last_b),
                        )
            csl = slice(J * CHUNK, (J + 1) * CHUNK)
            nc.vector.tensor_copy(oT01[0:64, csl], pv0[0:64, :])
            nc.vector.tensor_copy(oT01[64:128, csl], pv1[0:64, :])
            nc.vector.tensor_copy(den0[:, csl], pv0[64:65, :])
            nc.vector.tensor_copy(den1[:, csl], pv1[64:65, :])

    # ---- phase 3: per-head output projection (undivided), heads row-paired ----
    nc.sync.dma_start(dens[0:1, :], den0[:])
    nc.sync.dma_start(dens[1:2, :], den1[:])
    with tc.tile_pool(name="ph3sb", bufs=2) as p3s, \
         tc.tile_pool(name="ps_po", bufs=2, space="PSUM") as ps_po:
        for J in range(NCHUNK):
            csl = slice(J * CHUNK, (J + 1) * CHUNK)
            for dt_ in range(4):
                po0 = ps_po.tile([128, CHUNK], F32, tag="po0")
                po1 = ps_po.tile([128, CHUNK], F32, tag="po1")
                for (a, b) in _pieces(0):
                    for po, hh in ((po0, 0), (po1, 1)):
                        hsl = slice(hh * 64, (hh + 1) * 64)
                        nc.tensor.matmul(
                            po[:, a:b],
                            wo01_sb[hsl, dt_ * 128:(dt_ + 1) * 128],
                            oT01[hsl, J * CHUNK + a:J * CHUNK + b],
                            start=True, stop=True)
                for po, poT, cpy in ((po0, po0T, nc.vector.tensor_copy),
                                     (po1, po1T, nc.scalar.copy)):
                    posb = p3s.tile([128, CHUNK], F32, tag="posb", bufs=4)
                    cpy(posb[:], po[:])
                    nc.sync.dma_start(poT[dt_ * 128:(dt_ + 1) * 128, csl],
                                      posb[:])


_CACHED = None


def _build():
    global _CACHED
    if _CACHED is not None:
        return _CACHED
    nc = bacc.Bacc("TRN2", target_bir_lowering=False, debug=False,
                   enable_asserts=False, num_devices=NCORES)
    names = [
        ("xT", [D, S], BF16), ("wq_p", [128, 512], BF16),
        ("wk_p", [128, 512], BF16), ("wv_p", [128, 512], BF16),
        ("wo01", [128, 512], BF16),
        ("bqkv", [128, 3], F32), ("kbias", [128, 32], F32),
        ("trimask", [128, 128], BF16), ("ident2", [128, 64], BF16),
    ]
    aps = [nc.dram_tensor(n, sh, dt_, kind="ExternalInput").ap()
           for n, sh, dt_ in names]
    po0T = nc.dram_tensor("po0T", [D, S], F32, kind="ExternalOutput").ap()
    po1T = nc.dram_tensor("po1T", [D, S], F32, kind="ExternalOutput").ap()
    dens = nc.dram_tensor("dens", [2, S], F32, kind="ExternalOutput").ap()
    with tile.TileContext(nc) as tc, ExitStack() as ctx:
        _emit(nc, tc, ctx, aps + [po0T, po1T, dens])
    nc.compile()
    _CACHED = nc
    return nc


def _host_inputs(x, attention_mask, Wq, bq, Wk, bk, Wv, bv, Wo, bo):
    f = np.float32
    x = np.asarray(x, f)
    mask = np.asarray(attention_mask)
    Wq, Wk, Wv, Wo = (np.asarray(w, f) for w in (Wq, Wk, Wv, Wo))
    bq, bk, bv = (np.asarray(b_, f) for b_ in (bq, bk, bv))
    tri = np.triu(np.ones((128, 128), NPBF16))      # [k,q]: 1 where q >= k
    id2 = np.tile(np.eye(64, dtype=NPBF16), (2, 1))
    in_maps = []
    for c in range(NCORES):
        b = c // 4
        h0 = 2 * (c % 4)
        hsl = slice(64 * h0, 64 * h0 + 128)

        def pack_w(W):
            wt = W[hsl, :].T                        # [512, 128] = Wh^T
            return np.ascontiguousarray(
                wt.reshape(4, 128, 128).transpose(1, 0, 2)
                .reshape(128, 512).astype(NPBF16))

        wo_t = Wo[:, hsl].T.astype(NPBF16)           # [128, 512]
        kb = np.where(mask[b] != 0, f(0.0), f(NEG)).astype(f)
        in_maps.append({
            "xT": np.ascontiguousarray(x[b].T.astype(NPBF16)),
            "wq_p": pack_w(Wq), "wk_p": pack_w(Wk), "wv_p": pack_w(Wv),
            "wo01": np.ascontiguousarray(wo_t),
            "bqkv": np.ascontiguousarray(
                np.stack([bq[hsl], bk[hsl], bv[hsl]], axis=1)),
            "kbias": np.ascontiguousarray(kb.reshape(32, 128).T),
            "trimask": tri, "ident2": id2,
        })
    return in_maps


def _assemble(results, bo):
    out = np.zeros((B, S, D), np.float32)
    for c in range(NCORES):
        r = results[c]
        dens = r["dens"]
        part = r["po0T"] / dens[0:1, :] + r["po1T"] / dens[1:2, :]
        out[c // 4] += part.T
    out += np.asarray(bo, np.float32)
    return out


def kernel(**inputs) -> np.ndarray:
    nc = _build()
    in_maps = _host_inputs(**inputs)
    last_err = None
    for attempt in range(3):
        try:
            res = bass_utils.run_bass_kernel_spmd(
                nc, in_maps, core_ids=list(range(NCORES)))
            out = _assemble(res.results, inputs["bo"])
        except Exception as e:  # transient NRT/axon device errors
            last_err = e
            continue
        if np.isfinite(out).all():
            return out
        last_err = RuntimeError("non-finite output")
    raise last_err


def run_traced(inputs, **kwargs):
    """test.py helper: run with NTFF tracing, return (out, BassKernelResults)."""
    nc = _build()
    in_maps = _host_inputs(**inputs)
    res = bass_utils.run_bass_kernel_spmd(
        nc, in_maps, core_ids=list(range(NCORES)), trace=True, **kwargs)
    return _assemble(res.results, inputs["bo"]), res



# revision 9
# speedup vs baseline: 1.2729x; 1.2729x over previous
"""Causal self-attention (B=2, S=4096, D=512, H=8) on 8 Trainium2 cores.

Sharding: core c handles batch b = c//4 and heads {2*(c%4), 2*(c%4)+1}.

Design (v2): k-major flash-style attention with the exp() wall split across
TWO engines:
  - ScalarE computes exp natively (ACTIVATE, ~(N+352)/1.2 ns).
  - VectorE computes a one-instruction Schraudolph exp: writing
    int16(round(A*score + bias)) whose bit pattern IS the bf16 of
    2^(log2e*score/8 + delta): the exponent-bit trick computed directly in
    the >>16 scale.  Per-key exponent dither delta_r decorrelates the
    interpolation error; V rows (and the den ones-column) are pre-scaled by
    2^-delta_r on the host so the dither cancels exactly in PV.
A greedy ns-balancer assigns each score tile's exp (and the psum->sbuf
copies) to whichever of ACT/DVE is less loaded, so both engines run ~full
tilt alongside the TensorE stream.

Attention runs in 512-wide query chunks; projections for chunk J+1, the
output projection for chunk J-1, V transposes, and DMA are emitted as
background tasks interleaved between attention steps so PE never idles
(keeps the HAM clock at 2.4 GHz).  Denominators ride the PV matmul as a
65th 'ones' row; oT is divided on-device (reciprocal_approx_fast + gpsimd
partition broadcast) so the two heads fold into ONE output-projection pass
and the core writes a single [512, S] bf16 partial that the host sums.

PSUM budget (8 banks): pv0 pv1 | st x4 (score tiles, f32) | bg x2 (shared
by projections / V-transpose / out-projection).
"""

import sys

sys.path.insert(0, "/opt/trn_rl_repo")

from contextlib import ExitStack

import ml_dtypes
import numpy as np

import concourse.bass as bass
import concourse.tile as tile
from concourse import bacc, bass_utils, mybir

B, S, D = 2, 4096, 512
H, HD = 8, 64
NCORES = 8
F32 = mybir.dt.float32
BF16 = mybir.dt.bfloat16
I16 = mybir.dt.int16
EXP = mybir.ActivationFunctionType.Exp
IDENT = mybir.ActivationFunctionType.Identity
COPYF = mybir.ActivationFunctionType.Copy
MULT = mybir.AluOpType.mult
ADD = mybir.AluOpType.add
NPBF16 = ml_dtypes.bfloat16

CK = 512                      # query-chunk width
NCH = S // CK                 # 8
KBLK = 128                    # key block (partition dim)
KB_PER_CK = CK // KBLK        # 4
NEG = -1.0e30
LOG2E = 1.4426950408889634
A128 = 128 * LOG2E * 0.125    # DVE trick multiplier (raw-score units)
CSH = -0.045                  # Schraudolph shift
PHI = 0.6180339887498949


class Balancer:
    """Greedy ns-accounting across ACT and DVE for balanceable ops."""

    def __init__(self, nc):
        self.nc = nc
        self.ns = {"act": 2700.0, "dve": 0.0}  # ACT pays the exp table load

    def _cost(self, eng, w):
        return (w + 352) / 1.2 if eng == "act" else (w + 90) / 0.96

    def charge(self, eng, w):
        self.ns[eng] += self._cost(eng, w)

    def pick(self, w):
        eng = "act" if self.ns["act"] + self._cost("act", w) <= \
            self.ns["dve"] + self._cost("dve", w) else "dve"
        self.charge(eng, w)
        return eng


def _emit(nc, tc, ctx, io):
    (xT, wq_p, wk_p, wv_p, wo01d, bqkv, kb23, biasact, trimask, ident2,
     vscale, poT) = io

    bal = Balancer(nc)

    const = ctx.enter_context(tc.tile_pool(name="const", bufs=1))
    sb = ctx.enter_context(tc.tile_pool(name="sb", bufs=1))

    # ---- constants / weights ----
    wq_sb = const.tile([128, 512], BF16, tag="wq")
    wk_sb = const.tile([128, 512], BF16, tag="wk")
    wv_sb = const.tile([128, 512], BF16, tag="wv")
    wo_sb = const.tile([128, 512], BF16, tag="wo")
    bqkv_sb = const.tile([128, 3], F32, tag="bqkv")
    kb23_sb = const.tile([128, 32], F32, tag="kb23")
    bact_sb = const.tile([128, 32], F32, tag="bact")
    tri_sb = const.tile([128, 128], BF16, tag="tri")
    id2_sb = const.tile([128, 64], BF16, tag="id2")
    vsc_sb = const.tile([128, 1], F32, tag="vsc")
    for t, a in ((wq_sb, wq_p), (wk_sb, wk_p), (wv_sb, wv_p), (wo_sb, wo01d),
                 (bqkv_sb, bqkv), (kb23_sb, kb23), (bact_sb, biasact),
                 (tri_sb, trimask), (id2_sb, ident2), (vsc_sb, vscale)):
        nc.sync.dma_start(t[:], a[:])

    # ---- persistent SBUF ----
    kT = sb.tile([128, S], BF16, tag="kT")       # [2*64 hd, keys]
    v0 = sb.tile([128, 32 * 65], BF16, tag="v0")  # k-major V + ones col, h0
    v1 = sb.tile([128, 32 * 65], BF16, tag="v1")

    xin = ctx.enter_context(tc.tile_pool(name="xin", bufs=2))
    qp = ctx.enter_context(tc.tile_pool(name="qp", bufs=2))
    vtp = ctx.enter_context(tc.tile_pool(name="vtp", bufs=2))
    etp = ctx.enter_context(tc.tile_pool(name="etp", bufs=6))
    otp = ctx.enter_context(tc.tile_pool(name="otp", bufs=2))
    pop = ctx.enter_context(tc.tile_pool(name="pop", bufs=4))
    rdp = ctx.enter_context(tc.tile_pool(name="rdp", bufs=2))

    ps_pv = ctx.enter_context(tc.tile_pool(name="ps_pv", bufs=1, space="PSUM"))
    ps_st = ctx.enter_context(tc.tile_pool(name="ps_st", bufs=1, space="PSUM"))
    ps_bg = ctx.enter_context(tc.tile_pool(name="ps_bg", bufs=2, space="PSUM"))

    # ones columns of v0/v1 (scaled 2^-delta); written once, blocks fill later
    for vdst in (v0, v1):
        ones_col = vdst[:].rearrange("p (k c) -> p k c", c=65)[:, :, 64:65]
        nc.vector.tensor_copy(ones_col, vsc_sb[:].to_broadcast((128, 32, 1)))

    # ---------------- background task machinery ----------------
    bg_tasks = []

    def drain(n):
        for _ in range(min(n, len(bg_tasks))):
            bg_tasks.pop(0)()

    def copy_psum(dst_ap, src_ap, w, bias_col=None):
        """psum->sbuf evacuation on the less-loaded of ACT/DVE."""
        eng = bal.pick(w)
        if eng == "act":
            if bias_col is not None:
                nc.scalar.activation(dst_ap, src_ap, IDENT, bias=bias_col,
                                     scale=1.0)
            else:
                nc.scalar.copy(dst_ap, src_ap)
        else:
            if bias_col is not None:
                nc.vector.tensor_scalar_add(dst_ap, src_ap, bias_col)
            else:
                nc.vector.tensor_copy(dst_ap, src_ap)

    q_tiles = {}

    def emit_proj(J):
        """q/k/v projections for chunk J + V transpose to k-major."""
        xs = []
        for ks in range(4):
            xt = xin.tile([128, CK], BF16, tag=f"x{ks}")
            nc.sync.dma_start(
                xt[:], xT[ks * 128:(ks + 1) * 128, J * CK:(J + 1) * CK])
            xs.append(xt)
        qt = qp.tile([128, CK], BF16, tag="q")
        q_tiles[J] = qt
        vt = vtp.tile([128, CK], BF16, tag="v")
        csl = slice(J * CK, (J + 1) * CK)

        def mk_proj(w_sb, bcol, dst_ap):
            def f():
                ps = ps_bg.tile([128, CK], F32, tag="bg")
                for ks in range(4):
                    nc.tensor.matmul(ps[:], w_sb[:, ks * 128:(ks + 1) * 128],
                                     xs[ks][:], start=(ks == 0),
                                     stop=(ks == 3))
                copy_psum(dst_ap, ps[:], CK,
                          bias_col=bqkv_sb[:, bcol:bcol + 1])
            return f

        bg_tasks.append(mk_proj(wq_sb, 0, qt[:]))
        bg_tasks.append(mk_proj(wk_sb, 1, kT[:, csl]))
        bg_tasks.append(mk_proj(wv_sb, 2, vt[:]))

        def mk_vtrans(hh, vdst):
            def f():
                # own psum buffer per head: a shared bank would let head0's
                # DVE copy (bank read) overlap head1's PE transposes (bank
                # write) -> fatal PSUM collision
                ps = ps_bg.tile([128, CK], F32, tag="bg")
                tr = ps[:].bitcast(BF16)  # [128, 1024] bf16 view
                for i in range(4):
                    nc.tensor.transpose(
                        tr[:, i * 64:(i + 1) * 64],
                        vt[hh * 64:(hh + 1) * 64, i * KBLK:(i + 1) * KBLK],
                        id2_sb[hh * 64:(hh + 1) * 64, :])
                dst = vdst[:, (J * 4) * 65:(J * 4 + 4) * 65]
                dst = dst.rearrange("p (k c) -> p k c", c=65)[:, :, 0:64]
                nc.vector.tensor_scalar_mul(
                    dst, tr[:, 0:256].rearrange("p (k c) -> p k c", c=64),
                    vsc_sb[:, 0:1])
                bal.charge("dve", 256)
            return f
        bg_tasks.append(mk_vtrans(0, v0))
        bg_tasks.append(mk_vtrans(1, v1))

    def emit_outproj(J):
        """out-projection of chunk J: po = Wo01 @ (oT/den); DMA out.

        The reciprocal + divide read the pv psum tiles, so they are emitted
        INLINE (before the next chunk re-acquires pv); only the Wo matmuls
        and output DMA go to the background queue."""
        oT = otp.tile([128, CK], BF16, tag="oT")
        pv0t, pv1t = pv_tiles.pop(J)

        # den rows live in pv[64:65]; evacuate, approx-reciprocal, broadcast
        rdB = []
        for hh, pvt in ((0, pv0t), (1, pv1t)):
            den = rdp.tile([1, CK], F32, tag=f"den{hh}")
            nc.vector.tensor_copy(den[:], pvt[64:65, :])
            rd = rdp.tile([1, CK], F32, tag=f"rd{hh}")
            nc.vector.reciprocal_approx_fast(rd[:], den[:])
            bal.charge("dve", 2 * CK)
            rb = rdp.tile([64, CK], F32, tag=f"rdB{hh}")
            nc.gpsimd.partition_broadcast(rb[:], rd[:], channels=64)
            rdB.append(rb)
        for hh, pvt in ((0, pv0t), (1, pv1t)):
            hsl = slice(hh * 64, (hh + 1) * 64)
            nc.vector.tensor_mul(oT[hsl, :], pvt[0:64, :], rdB[hh][:])
            bal.charge("dve", CK)

        def mk_dblk(dt_):
            def f():
                ps = ps_bg.tile([128, CK], F32, tag="bg")
                nc.tensor.matmul(ps[:], wo_sb[:, dt_ * 128:(dt_ + 1) * 128],
                                 oT[:], start=True, stop=True)
                po = pop.tile([128, CK], BF16, tag="po")
                copy_psum(po[:], ps[:], CK)
                nc.sync.dma_start(
                    poT[dt_ * 128:(dt_ + 1) * 128, J * CK:(J + 1) * CK],
                    po[:])
            return f
        for dt_ in range(4):
            bg_tasks.append(mk_dblk(dt_))

    # ---------------- main pipeline ----------------
    pv_tiles = {}
    emit_proj(0)
    drain(99)  # chunk 0 projections up front

    for J in range(NCH):
        if J + 1 < NCH:
            emit_proj(J + 1)
        pv0t = ps_pv.tile([65, CK], F32, tag="pv0")
        pv1t = ps_pv.tile([65, CK], F32, tag="pv1")
        pv_tiles[J] = (pv0t, pv1t)
        nkb = KB_PER_CK * (J + 1)
        qt = q_tiles.pop(J)

        for kb in range(nkb):
            p = kb - KB_PER_CK * J
            col0 = KBLK * p if p >= 0 else 0
            w = CK - col0
            sts, ets = [], []
            for hh in range(2):
                st = ps_st.tile([128, CK], F32, tag=f"st{hh}{kb % 2}")
                hsl = slice(hh * 64, (hh + 1) * 64)
                nc.tensor.matmul(
                    st[:, col0:], kT[hsl, kb * KBLK:(kb + 1) * KBLK],
                    qt[hsl, col0:], start=True, stop=True)
                sts.append(st)
            drain(1)
            for hh in range(2):
                et = etp.tile([128, CK], BF16, tag="et")
                eng = bal.pick(w)
                if eng == "act":
                    nc.scalar.activation(
                        et[:, col0:], sts[hh][:, col0:], EXP,
                        bias=bact_sb[:, kb:kb + 1], scale=0.125)
                else:
                    nc.vector.scalar_tensor_tensor(
                        et[:, col0:].bitcast(I16), sts[hh][:, col0:],
                        float(A128),
                        kb23_sb[:, kb:kb + 1].to_broadcast((128, w)),
                        op0=MULT, op1=ADD)
                if p >= 0:
                    nc.vector.tensor_mul(
                        et[:, col0:col0 + KBLK], et[:, col0:col0 + KBLK],
                        tri_sb[:])
                    bal.charge("dve", KBLK / 2)
                ets.append(et)
            drain(1)
            for hh, vsb, pv in ((0, v0, pv0t), (1, v1, pv1t)):
                nc.tensor.matmul(
                    pv[:, col0:], vsb[:, kb * 65:(kb + 1) * 65],
                    ets[hh][:, col0:],
                    start=(kb == 0), stop=(kb == nkb - 1))
        emit_outproj(J)
        drain(2)

    drain(99)


_CACHED = None


def _build():
    global _CACHED
    if _CACHED is not None:
        return _CACHED
    nc = bacc.Bacc("TRN2", target_bir_lowering=False, debug=False,
                   enable_asserts=False, num_devices=NCORES)
    names = [
        ("xT", [D, S], BF16), ("wq_p", [128, 512], BF16),
        ("wk_p", [128, 512], BF16), ("wv_p", [128, 512], BF16),
        ("wo01", [128, 512], BF16), ("bqkv", [128, 3], F32),
        ("kb23", [128, 32], F32), ("biasact", [128, 32], F32),
        ("trimask", [128, 128], BF16), ("ident2", [128, 64], BF16),
        ("vscale", [128, 1], F32),
    ]
    aps = [nc.dram_tensor(n, sh, dt_, kind="ExternalInput").ap()
           for n, sh, dt_ in names]
    poT = nc.dram_tensor("poT", [D, S], BF16, kind="ExternalOutput").ap()
    with tile.TileContext(nc) as tc, ExitStack() as ctx:
        _emit(nc, tc, ctx, aps + [poT])
    nc.compile()
    _CACHED = nc
    return nc


def _host_inputs(x, attention_mask, Wq, bq, Wk, bk, Wv, bv, Wo, bo):
    f = np.float32
    x = np.asarray(x, f)
    mask = np.asarray(attention_mask)
    Wq, Wk, Wv, Wo = (np.asarray(w, f) for w in (Wq, Wk, Wv, Wo))
    bq, bk, bv = (np.asarray(b_, f) for b_ in (bq, bk, bv))
    tri = np.triu(np.ones((128, 128), NPBF16))      # [k,q]: 1 where q >= k
    id2 = np.tile(np.eye(64, dtype=NPBF16), (2, 1))
    delta = ((np.arange(128) * PHI) % 1.0).astype(f)          # per key%128
    vscale = (2.0 ** -delta)[:, None].astype(f)
    in_maps = []
    for c in range(NCORES):
        b = c // 4
        h0 = 2 * (c % 4)
        hsl = slice(64 * h0, 64 * h0 + 128)

        def pack_w(W):
            wt = W[hsl, :].T                        # [512, 128] = Wh^T
            return np.ascontiguousarray(
                wt.reshape(4, 128, 128).transpose(1, 0, 2)
                .reshape(128, 512).astype(NPBF16))

        wo_t = Wo[:, hsl].T.astype(NPBF16)           # [128, 512]
        mk = np.where(mask[b] != 0, f(0.0), f(NEG)).astype(f)  # [S]
        mk = mk.reshape(32, 128).T                   # [128 part, 32 kb]
        kb23 = (128.0 * (127.0 + CSH) + 128.0 * delta)[:, None] + \
            np.where(mk < 0, f(-1e9), f(0.0))
        biasact = (delta * np.log(2.0))[:, None] + mk
        in_maps.append({
            "xT": np.ascontiguousarray(x[b].T.astype(NPBF16)),
            "wq_p": pack_w(Wq), "wk_p": pack_w(Wk), "wv_p": pack_w(Wv),
            "wo01": np.ascontiguousarray(wo_t),
            "bqkv": np.ascontiguousarray(
                np.stack([bq[hsl], bk[hsl], bv[hsl]], axis=1)),
            "kb23": np.ascontiguousarray(kb23.astype(f)),
            "biasact": np.ascontiguousarray(biasact.astype(f)),
            "trimask": tri, "ident2": id2, "vscale": vscale,
        })
    return in_maps


def _assemble(results, bo):
    out = np.zeros((B, S, D), np.float32)
    for c in range(NCORES):
        out[c // 4] += results[c]["poT"].astype(np.float32).T
    out += np.asarray(bo, np.float32)
    return out


def kernel(**inputs) -> np.ndarray:
    nc = _build()
    in_maps = _host_inputs(**inputs)
    last_err = None
    for attempt in range(3):
        try:
            res = bass_utils.run_bass_kernel_spmd(
                nc, in_maps, core_ids=list(range(NCORES)))
            out = _assemble(res.results, inputs["bo"])
        except Exception as e:  # transient NRT/axon device errors
            last_err = e
            continue
        if np.isfinite(out).all():
            return out
        last_err = RuntimeError("non-finite output")
    raise last_err


def run_traced(inputs, **kwargs):
    """test.py helper: run with NTFF tracing, return (out, BassKernelResults)."""
    nc = _build()
    in_maps = _host_inputs(**inputs)
    res = bass_utils.run_bass_kernel_spmd(
        nc, in_maps, core_ids=list(range(NCORES)), trace=True, **kwargs)
    return _assemble(res.results, inputs["bo"]), res


# revision 12
# speedup vs baseline: 1.3181x; 1.0355x over previous
"""Causal self-attention (B=2, S=4096, D=512, H=8) on 8 Trainium2 cores.

Sharding: core c handles batch b = c//4 and heads {2*(c%4), 2*(c%4)+1}.

Design (v2): k-major flash-style attention with the exp() wall split across
TWO engines:
  - ScalarE computes exp natively (ACTIVATE, ~(N+352)/1.2 ns).
  - VectorE computes a one-instruction Schraudolph exp: writing
    int16(round(A*score + bias)) whose bit pattern IS the bf16 of
    2^(log2e*score/8 + delta): the exponent-bit trick computed directly in
    the >>16 scale.  Per-key exponent dither delta_r decorrelates the
    interpolation error; V rows (and the den ones-column) are pre-scaled by
    2^-delta_r on the host so the dither cancels exactly in PV.
A greedy ns-balancer assigns each score tile's exp (and the psum->sbuf
copies) to whichever of ACT/DVE is less loaded, so both engines run ~full
tilt alongside the TensorE stream.

Attention runs in 512-wide query chunks; projections for chunk J+1, the
output projection for chunk J-1, V transposes, and DMA are emitted as
background tasks interleaved between attention steps so PE never idles
(keeps the HAM clock at 2.4 GHz).  Denominators ride the PV matmul as a
65th 'ones' row; oT is divided on-device (reciprocal_approx_fast + gpsimd
partition broadcast) so the two heads fold into ONE output-projection pass
and the core writes a single [512, S] bf16 partial that the host sums.

PSUM budget (8 banks): pv0 pv1 | st x4 (score tiles, f32) | bg x2 (shared
by projections / V-transpose / out-projection).
"""

import sys

sys.path.insert(0, "/opt/trn_rl_repo")

from contextlib import ExitStack

import ml_dtypes
import numpy as np

import concourse.bass as bass
import concourse.tile as tile
from concourse import bacc, bass_utils, mybir

B, S, D = 2, 4096, 512
H, HD = 8, 64
NCORES = 8
F32 = mybir.dt.float32
BF16 = mybir.dt.bfloat16
I16 = mybir.dt.int16
EXP = mybir.ActivationFunctionType.Exp
IDENT = mybir.ActivationFunctionType.Identity
COPYF = mybir.ActivationFunctionType.Copy
MULT = mybir.AluOpType.mult
ADD = mybir.AluOpType.add
NPBF16 = ml_dtypes.bfloat16

CK = 512                      # query-chunk width
NCH = S // CK                 # 8
KBLK = 128                    # key block (partition dim)
KB_PER_CK = CK // KBLK        # 4
NEG = -1.0e30
LOG2E = 1.4426950408889634
A128 = 128 * LOG2E * 0.125    # DVE trick multiplier (raw-score units)
CSH = -0.045                  # Schraudolph shift
PHI = 0.6180339887498949


class Balancer:
    """Greedy ns-accounting across ACT and DVE for balanceable ops."""

    def __init__(self, nc):
        self.nc = nc
        self.ns = {"act": 2700.0, "dve": 0.0}  # ACT pays the exp table load

    def _cost(self, eng, w):
        return (w + 352) / 1.2 if eng == "act" else (w + 90) / 0.96

    def charge(self, eng, w):
        self.ns[eng] += self._cost(eng, w)

    def pick(self, w):
        eng = "act" if self.ns["act"] + self._cost("act", w) <= \
            self.ns["dve"] + self._cost("dve", w) else "dve"
        self.charge(eng, w)
        return eng


def _emit(nc, tc, ctx, io):
    (xT, wq_p, wk_p, wv_p, wo01d, bqkv, kb23, biasact, trimask, ident2,
     vscale, poT) = io

    bal = Balancer(nc)

    const = ctx.enter_context(tc.tile_pool(name="const", bufs=1))
    sb = ctx.enter_context(tc.tile_pool(name="sb", bufs=1))

    # ---- constants / weights ----
    wq_sb = const.tile([128, 512], BF16, tag="wq")
    wk_sb = const.tile([128, 512], BF16, tag="wk")
    wv_sb = const.tile([128, 512], BF16, tag="wv")
    wo_sb = const.tile([128, 512], BF16, tag="wo")
    bqkv_sb = const.tile([128, 3], F32, tag="bqkv")
    kb23_sb = const.tile([128, 32], F32, tag="kb23")
    bact_sb = const.tile([128, 32], F32, tag="bact")
    tri_sb = const.tile([128, 128], BF16, tag="tri")
    id2_sb = const.tile([128, 64], BF16, tag="id2")
    vsc_sb = const.tile([128, 1], F32, tag="vsc")
    for t, a in ((wq_sb, wq_p), (wk_sb, wk_p), (wv_sb, wv_p), (wo_sb, wo01d),
                 (bqkv_sb, bqkv), (kb23_sb, kb23), (bact_sb, biasact),
                 (tri_sb, trimask), (id2_sb, ident2), (vsc_sb, vscale)):
        nc.sync.dma_start(t[:], a[:])

    # ---- persistent SBUF ----
    kT = sb.tile([128, S], BF16, tag="kT")       # [2*64 hd, keys]
    v0 = sb.tile([128, 32 * 65], BF16, tag="v0")  # k-major V + ones col, h0
    v1 = sb.tile([128, 32 * 65], BF16, tag="v1")

    xin = ctx.enter_context(tc.tile_pool(name="xin", bufs=2))
    qp = ctx.enter_context(tc.tile_pool(name="qp", bufs=2))
    vtp = ctx.enter_context(tc.tile_pool(name="vtp", bufs=2))
    etp = ctx.enter_context(tc.tile_pool(name="etp", bufs=6))
    otp = ctx.enter_context(tc.tile_pool(name="otp", bufs=2))
    pop = ctx.enter_context(tc.tile_pool(name="pop", bufs=4))
    rdp = ctx.enter_context(tc.tile_pool(name="rdp", bufs=2))

    ps_pv = ctx.enter_context(tc.tile_pool(name="ps_pv", bufs=1, space="PSUM"))
    ps_st = ctx.enter_context(tc.tile_pool(name="ps_st", bufs=1, space="PSUM"))
    ps_bg = ctx.enter_context(tc.tile_pool(name="ps_bg", bufs=2, space="PSUM"))

    # ones columns of v0/v1 (scaled 2^-delta); written once, blocks fill later
    for vdst in (v0, v1):
        ones_col = vdst[:].rearrange("p (k c) -> p k c", c=65)[:, :, 64:65]
        nc.vector.tensor_copy(ones_col, vsc_sb[:].to_broadcast((128, 32, 1)))

    # ---------------- background task machinery ----------------
    bg_tasks = []

    def drain(n):
        for _ in range(min(n, len(bg_tasks))):
            bg_tasks.pop(0)()

    def copy_psum(dst_ap, src_ap, w, bias_col=None):
        """psum->sbuf evacuation on the less-loaded of ACT/DVE."""
        eng = bal.pick(w)
        if eng == "act":
            if bias_col is not None:
                nc.scalar.activation(dst_ap, src_ap, IDENT, bias=bias_col,
                                     scale=1.0)
            else:
                nc.scalar.copy(dst_ap, src_ap)
        else:
            if bias_col is not None:
                nc.vector.tensor_scalar_add(dst_ap, src_ap, bias_col)
            else:
                nc.vector.tensor_copy(dst_ap, src_ap)

    q_tiles = {}

    def emit_proj(J):
        """q/k/v projections for chunk J + V transpose to k-major."""
        xs = []
        for ks in range(4):
            xt = xin.tile([128, CK], BF16, tag=f"x{ks}")
            nc.sync.dma_start(
                xt[:], xT[ks * 128:(ks + 1) * 128, J * CK:(J + 1) * CK])
            xs.append(xt)
        qt = qp.tile([128, CK], BF16, tag="q")
        q_tiles[J] = qt
        vt = vtp.tile([128, CK], BF16, tag="v")
        csl = slice(J * CK, (J + 1) * CK)

        def mk_proj(w_sb, bcol, dst_ap):
            def f():
                ps = ps_bg.tile([128, CK], F32, tag="bg")
                for ks in range(4):
                    nc.tensor.matmul(ps[:], w_sb[:, ks * 128:(ks + 1) * 128],
                                     xs[ks][:], start=(ks == 0),
                                     stop=(ks == 3))
                copy_psum(dst_ap, ps[:], CK,
                          bias_col=bqkv_sb[:, bcol:bcol + 1])
            return f

        bg_tasks.append(mk_proj(wq_sb, 0, qt[:]))
        bg_tasks.append(mk_proj(wk_sb, 1, kT[:, csl]))
        bg_tasks.append(mk_proj(wv_sb, 2, vt[:]))

        def mk_vtrans(hh, vdst):
            def f():
                # own psum buffer per head: a shared bank would let head0's
                # DVE copy (bank read) overlap head1's PE transposes (bank
                # write) -> fatal PSUM collision
                ps = ps_bg.tile([128, CK], F32, tag="bg")
                tr = ps[:].bitcast(BF16)  # [128, 1024] bf16 view
                for i in range(4):
                    nc.tensor.transpose(
                        tr[:, i * 64:(i + 1) * 64],
                        vt[hh * 64:(hh + 1) * 64, i * KBLK:(i + 1) * KBLK],
                        id2_sb[hh * 64:(hh + 1) * 64, :])
                dst = vdst[:, (J * 4) * 65:(J * 4 + 4) * 65]
                dst = dst.rearrange("p (k c) -> p k c", c=65)[:, :, 0:64]
                nc.vector.tensor_scalar_mul(
                    dst, tr[:, 0:256].rearrange("p (k c) -> p k c", c=64),
                    vsc_sb[:, 0:1])
                bal.charge("dve", 256)
            return f
        bg_tasks.append(mk_vtrans(0, v0))
        bg_tasks.append(mk_vtrans(1, v1))

    oT_tiles = {}

    def emit_div(J):
        """INLINE at chunk-J end: den reciprocal + broadcast + oT divide.
        Reads the pv psum tiles, so must precede the next pv acquisition."""
        oT = otp.tile([128, CK], BF16, tag="oT")
        oT_tiles[J] = oT
        pv0t, pv1t = pv_tiles.pop(J)
        rdB = []
        for hh, pvt in ((0, pv0t), (1, pv1t)):
            den = rdp.tile([1, CK], F32, tag=f"den{hh}")
            nc.vector.tensor_copy(den[:], pvt[64:65, :])
            rd = rdp.tile([1, CK], F32, tag=f"rd{hh}")
            nc.vector.reciprocal_approx_fast(rd[:], den[:])
            bal.charge("dve", 2 * CK)
            rb = rdp.tile([64, CK], F32, tag=f"rdB{hh}")
            nc.gpsimd.partition_broadcast(rb[:], rd[:], channels=64)
            rdB.append(rb)
        for hh, pvt in ((0, pv0t), (1, pv1t)):
            hsl = slice(hh * 64, (hh + 1) * 64)
            nc.vector.tensor_mul(oT[hsl, :], pvt[0:64, :], rdB[hh][:])
            bal.charge("dve", CK)

    def emit_outproj(J):
        """Queue chunk J's Wo matmuls + output DMA (oT(J) long ready by the
        time these drain, so they never block the PE FIFO)."""
        oT = oT_tiles.pop(J)

        def mk_dblk(dt_):
            def f():
                ps = ps_bg.tile([128, CK], F32, tag="bg")
                nc.tensor.matmul(ps[:], wo_sb[:, dt_ * 128:(dt_ + 1) * 128],
                                 oT[:], start=True, stop=True)
                po = pop.tile([128, CK], BF16, tag="po")
                copy_psum(po[:], ps[:], CK)
                nc.sync.dma_start(
                    poT[dt_ * 128:(dt_ + 1) * 128, J * CK:(J + 1) * CK],
                    po[:])
            return f
        for dt_ in range(4):
            bg_tasks.append(mk_dblk(dt_))

    # ---------------- main pipeline ----------------
    pv_tiles = {}
    emit_proj(0)
    drain(99)  # chunk 0 projections up front

    for J in range(NCH):
        if J + 1 < NCH:
            emit_proj(J + 1)
        if J >= 1:
            emit_outproj(J - 1)  # behind proj(J+1) tasks: oT(J-1) is ready
        pv0t = ps_pv.tile([65, CK], F32, tag="pv0")
        pv1t = ps_pv.tile([65, CK], F32, tag="pv1")
        pv_tiles[J] = (pv0t, pv1t)
        nkb = KB_PER_CK * (J + 1)
        qt = q_tiles.pop(J)

        for kb in range(nkb):
            p = kb - KB_PER_CK * J
            col0 = KBLK * p if p >= 0 else 0
            w = CK - col0
            sts, ets = [], []
            for hh in range(2):
                st = ps_st.tile([128, CK], F32, tag=f"st{hh}{kb % 2}")
                hsl = slice(hh * 64, (hh + 1) * 64)
                nc.tensor.matmul(
                    st[:, col0:], kT[hsl, kb * KBLK:(kb + 1) * KBLK],
                    qt[hsl, col0:], start=True, stop=True)
                sts.append(st)
            drain(1)
            for hh in range(2):
                et = etp.tile([128, CK], BF16, tag="et")
                eng = bal.pick(w)
                if eng == "act":
                    nc.scalar.activation(
                        et[:, col0:], sts[hh][:, col0:], EXP,
                        bias=bact_sb[:, kb:kb + 1], scale=0.125)
                else:
                    nc.vector.scalar_tensor_tensor(
                        et[:, col0:].bitcast(I16), sts[hh][:, col0:],
                        float(A128),
                        kb23_sb[:, kb:kb + 1].to_broadcast((128, w)),
                        op0=MULT, op1=ADD)
                if p >= 0:
                    nc.vector.tensor_mul(
                        et[:, col0:col0 + KBLK], et[:, col0:col0 + KBLK],
                        tri_sb[:])
                    bal.charge("dve", KBLK / 2)
                ets.append(et)
            drain(1)
            for hh, vsb, pv in ((0, v0, pv0t), (1, v1, pv1t)):
                nc.tensor.matmul(
                    pv[:, col0:], vsb[:, kb * 65:(kb + 1) * 65],
                    ets[hh][:, col0:],
                    start=(kb == 0), stop=(kb == nkb - 1))
        emit_div(J)
        drain(2)

    emit_outproj(NCH - 1)
    drain(99)


_CACHED = None


def _build():
    global _CACHED
    if _CACHED is not None:
        return _CACHED
    nc = bacc.Bacc("TRN2", target_bir_lowering=False, debug=False,
                   enable_asserts=False, num_devices=NCORES)
    names = [
        ("xT", [D, S], BF16), ("wq_p", [128, 512], BF16),
        ("wk_p", [128, 512], BF16), ("wv_p", [128, 512], BF16),
        ("wo01", [128, 512], BF16), ("bqkv", [128, 3], F32),
        ("kb23", [128, 32], F32), ("biasact", [128, 32], F32),
        ("trimask", [128, 128], BF16), ("ident2", [128, 64], BF16),
        ("vscale", [128, 1], F32),
    ]
    aps = [nc.dram_tensor(n, sh, dt_, kind="ExternalInput").ap()
           for n, sh, dt_ in names]
    poT = nc.dram_tensor("poT", [D, S], BF16, kind="ExternalOutput").ap()
    with tile.TileContext(nc) as tc, ExitStack() as ctx:
        _emit(nc, tc, ctx, aps + [poT])
    nc.compile()
    _CACHED = nc
    return nc


def _host_inputs(x, attention_mask, Wq, bq, Wk, bk, Wv, bv, Wo, bo):
    f = np.float32
    x = np.asarray(x, f)
    mask = np.asarray(attention_mask)
    Wq, Wk, Wv, Wo = (np.asarray(w, f) for w in (Wq, Wk, Wv, Wo))
    bq, bk, bv = (np.asarray(b_, f) for b_ in (bq, bk, bv))
    tri = np.triu(np.ones((128, 128), NPBF16))      # [k,q]: 1 where q >= k
    id2 = np.tile(np.eye(64, dtype=NPBF16), (2, 1))
    delta = ((np.arange(128) * PHI) % 1.0).astype(f)          # per key%128
    vscale = (2.0 ** -delta)[:, None].astype(f)
    in_maps = []
    for c in range(NCORES):
        b = c // 4
        h0 = 2 * (c % 4)
        hsl = slice(64 * h0, 64 * h0 + 128)

        def pack_w(W):
            wt = W[hsl, :].T                        # [512, 128] = Wh^T
            return np.ascontiguousarray(
                wt.reshape(4, 128, 128).transpose(1, 0, 2)
                .reshape(128, 512).astype(NPBF16))

        wo_t = Wo[:, hsl].T.astype(NPBF16)           # [128, 512]
        mk = np.where(mask[b] != 0, f(0.0), f(NEG)).astype(f)  # [S]
        mk = mk.reshape(32, 128).T                   # [128 part, 32 kb]
        kb23 = (128.0 * (127.0 + CSH) + 128.0 * delta)[:, None] + \
            np.where(mk < 0, f(-1e9), f(0.0))
        biasact = (delta * np.log(2.0))[:, None] + mk
        in_maps.append({
            "xT": np.ascontiguousarray(x[b].T.astype(NPBF16)),
            "wq_p": pack_w(Wq), "wk_p": pack_w(Wk), "wv_p": pack_w(Wv),
            "wo01": np.ascontiguousarray(wo_t),
            "bqkv": np.ascontiguousarray(
                np.stack([bq[hsl], bk[hsl], bv[hsl]], axis=1)),
            "kb23": np.ascontiguousarray(kb23.astype(f)),
            "biasact": np.ascontiguousarray(biasact.astype(f)),
            "trimask": tri, "ident2": id2, "vscale": vscale,
        })
    return in_maps


def _assemble(results, bo):
    out = np.zeros((B, S, D), np.float32)
    for c in range(NCORES):
        out[c // 4] += results[c]["poT"].astype(np.float32).T
    out += np.asarray(bo, np.float32)
    return out


def kernel(**inputs) -> np.ndarray:
    nc = _build()
    in_maps = _host_inputs(**inputs)
    last_err = None
    for attempt in range(3):
        try:
            res = bass_utils.run_bass_kernel_spmd(
                nc, in_maps, core_ids=list(range(NCORES)))
            out = _assemble(res.results, inputs["bo"])
        except Exception as e:  # transient NRT/axon device errors
            last_err = e
            continue
        if np.isfinite(out).all():
            return out
        last_err = RuntimeError("non-finite output")
    raise last_err


def run_traced(inputs, **kwargs):
    """test.py helper: run with NTFF tracing, return (out, BassKernelResults)."""
    nc = _build()
    in_maps = _host_inputs(**inputs)
    res = bass_utils.run_bass_kernel_spmd(
        nc, in_maps, core_ids=list(range(NCORES)), trace=True, **kwargs)
    return _assemble(res.results, inputs["bo"]), res


# revision 20
# speedup vs baseline: 1.4234x; 1.0799x over previous
"""Causal self-attention (B=2, S=4096, D=512, H=8) on 8 Trainium2 cores.

Sharding: core c handles batch b = c//4 and heads {2*(c%4), 2*(c%4)+1}.

Design (v2): k-major flash-style attention with the exp() wall split across
TWO engines:
  - ScalarE computes exp natively (ACTIVATE, ~(N+352)/1.2 ns).
  - VectorE computes a one-instruction Schraudolph exp: writing
    int16(round(A*score + bias)) whose bit pattern IS the bf16 of
    2^(log2e*score/8 + delta): the exponent-bit trick computed directly in
    the >>16 scale.  Per-key exponent dither delta_r decorrelates the
    interpolation error; V rows (and the den ones-column) are pre-scaled by
    2^-delta_r on the host so the dither cancels exactly in PV.
A greedy ns-balancer assigns each score tile's exp (and the psum->sbuf
copies) to whichever of ACT/DVE is less loaded, so both engines run ~full
tilt alongside the TensorE stream.

Attention runs in 512-wide query chunks; projections for chunk J+1, the
output projection for chunk J-1, V transposes, and DMA are emitted as
background tasks interleaved between attention steps so PE never idles
(keeps the HAM clock at 2.4 GHz).  Denominators ride the PV matmul as a
65th 'ones' row; oT is divided on-device (reciprocal_approx_fast + gpsimd
partition broadcast) so the two heads fold into ONE output-projection pass
and the core writes a single [512, S] bf16 partial that the host sums.

PSUM budget (8 banks): pv0 pv1 | st x4 (score tiles, f32) | bg x2 (shared
by projections / V-transpose / out-projection).
"""

import sys

sys.path.insert(0, "/opt/trn_rl_repo")

from contextlib import ExitStack

import ml_dtypes
import numpy as np

import concourse.bass as bass
import concourse.tile as tile
from concourse import bacc, bass_utils, mybir

B, S, D = 2, 4096, 512
H, HD = 8, 64
NCORES = 8
F32 = mybir.dt.float32
BF16 = mybir.dt.bfloat16
I16 = mybir.dt.int16
EXP = mybir.ActivationFunctionType.Exp
IDENT = mybir.ActivationFunctionType.Identity
COPYF = mybir.ActivationFunctionType.Copy
MULT = mybir.AluOpType.mult
ADD = mybir.AluOpType.add
NPBF16 = ml_dtypes.bfloat16

CK = 512                      # query-chunk width
NCH = S // CK                 # 8
KBLK = 128                    # key block (partition dim)
KB_PER_CK = CK // KBLK        # 4
NEG = -1.0e30
LOG2E = 1.4426950408889634
A128 = 128 * LOG2E * 0.125    # DVE trick multiplier (raw-score units)
CSH = -0.045                  # Schraudolph shift
PHI = 0.6180339887498949


class Balancer:
    """Greedy ns-accounting across ACT and DVE for balanceable ops."""

    def __init__(self, nc):
        self.nc = nc
        self.ns = {"act": 2700.0, "dve": 0.0}  # ACT pays the exp table load

    def _cost(self, eng, w):
        return (w + 352) / 1.2 if eng == "act" else (w + 90) / 0.96

    def charge(self, eng, w):
        self.ns[eng] += self._cost(eng, w)

    def pick(self, w):
        eng = "act" if self.ns["act"] + self._cost("act", w) <= \
            self.ns["dve"] + self._cost("dve", w) else "dve"
        self.charge(eng, w)
        return eng


def _emit(nc, tc, ctx, io):
    xT, wpack, fpack, poT = io

    bal = Balancer(nc)

    const = ctx.enter_context(tc.tile_pool(name="const", bufs=1))
    sb = ctx.enter_context(tc.tile_pool(name="sb", bufs=1))

    # ---- constants / weights (two packed DMAs to keep the lead-in short) ----
    cb = const.tile([128, 2240], BF16, tag="cbf16")
    cf = const.tile([128, 68], F32, tag="cf32")
    nc.sync.dma_start(cb[:], wpack[:])
    nc.sync.dma_start(cf[:], fpack[:])
    W_Q, W_K, W_V, W_O, TRI, ID2 = 0, 512, 1024, 1536, 2048, 2176
    BQKV, KB23, BACT, VSC = 0, 3, 35, 67

    # ---- persistent SBUF ----
    kT = sb.tile([128, S], BF16, tag="kT")       # [2*64 hd, keys]
    v0 = sb.tile([128, 32 * 65], BF16, tag="v0")  # k-major V + ones col, h0
    v1 = sb.tile([128, 32 * 65], BF16, tag="v1")

    xin = ctx.enter_context(tc.tile_pool(name="xin", bufs=2))
    qp = ctx.enter_context(tc.tile_pool(name="qp", bufs=2))
    vtp = ctx.enter_context(tc.tile_pool(name="vtp", bufs=2))
    etp = ctx.enter_context(tc.tile_pool(name="etp", bufs=6))
    otp = ctx.enter_context(tc.tile_pool(name="otp", bufs=2))
    pop = ctx.enter_context(tc.tile_pool(name="pop", bufs=4))
    rdp = ctx.enter_context(tc.tile_pool(name="rdp", bufs=2))

    ps_pv = ctx.enter_context(tc.tile_pool(name="ps_pv", bufs=1, space="PSUM"))
    ps_st = ctx.enter_context(tc.tile_pool(name="ps_st", bufs=1, space="PSUM"))
    ps_bg = ctx.enter_context(tc.tile_pool(name="ps_bg", bufs=2, space="PSUM"))

    # ones columns of v0/v1 (scaled 2^-delta); written once, blocks fill later
    for vdst in (v0, v1):
        ones_col = vdst[:].rearrange("p (k c) -> p k c", c=65)[:, :, 64:65]
        nc.vector.tensor_copy(ones_col, cf[:, VSC:VSC + 1].to_broadcast((128, 32, 1)))

    # ---------------- background task machinery ----------------
    bg_tasks = []
    pace = {"credit": 0.0, "rate": 1.0}

    def drain(n):
        for _ in range(min(n, len(bg_tasks))):
            bg_tasks.pop(0)()

    def drain_paced():
        """Spread queued tasks over the chunk's drain slots so the PE always
        has background work, even late in a chunk."""
        pace["credit"] += pace["rate"]
        while pace["credit"] >= 1.0 and bg_tasks:
            pace["credit"] -= 1.0
            bg_tasks.pop(0)()

    def copy_psum(dst_ap, src_ap, w, bias_col=None):
        """psum->sbuf evacuation on the less-loaded of ACT/DVE."""
        eng = bal.pick(w)
        if eng == "act":
            if bias_col is not None:
                nc.scalar.activation(dst_ap, src_ap, IDENT, bias=bias_col,
                                     scale=1.0)
            else:
                nc.scalar.copy(dst_ap, src_ap)
        else:
            if bias_col is not None:
                nc.vector.tensor_scalar_add(dst_ap, src_ap, bias_col)
            else:
                nc.vector.tensor_copy(dst_ap, src_ap)

    q_tiles = {}

    def emit_proj(J):
        """q/k/v projections for chunk J + V transpose to k-major."""
        xs = []
        for ks in range(4):
            xt = xin.tile([128, CK], BF16, tag=f"x{ks}")
            nc.sync.dma_start(
                xt[:], xT[ks * 128:(ks + 1) * 128, J * CK:(J + 1) * CK])
            xs.append(xt)
        qt = qp.tile([128, CK], BF16, tag="q")
        q_tiles[J] = qt
        vt = vtp.tile([128, CK], BF16, tag="v")
        csl = slice(J * CK, (J + 1) * CK)

        def mk_proj(woff, bcol, dst_ap):
            def f():
                ps = ps_bg.tile([128, CK], F32, tag="bg")
                for ks in range(4):
                    nc.tensor.matmul(ps[:], cb[:, woff + ks * 128:woff + (ks + 1) * 128],
                                     xs[ks][:], start=(ks == 0),
                                     stop=(ks == 3))
                copy_psum(dst_ap, ps[:], CK,
                          bias_col=cf[:, BQKV + bcol:BQKV + bcol + 1])
            return f

        bg_tasks.append(mk_proj(W_Q, 0, qt[:]))
        bg_tasks.append(mk_proj(W_K, 1, kT[:, csl]))
        bg_tasks.append(mk_proj(W_V, 2, vt[:]))

        def mk_vtrans(hh, vdst):
            def f():
                # own psum buffer per head: a shared bank would let head0's
                # DVE copy (bank read) overlap head1's PE transposes (bank
                # write) -> fatal PSUM collision
                ps = ps_bg.tile([128, CK], F32, tag="bg")
                tr = ps[:].bitcast(BF16)  # [128, 1024] bf16 view
                for i in range(4):
                    nc.tensor.transpose(
                        tr[:, i * 64:(i + 1) * 64],
                        vt[hh * 64:(hh + 1) * 64, i * KBLK:(i + 1) * KBLK],
                        cb[hh * 64:(hh + 1) * 64, ID2:ID2 + 64])
                dst = vdst[:, (J * 4) * 65:(J * 4 + 4) * 65]
                dst = dst.rearrange("p (k c) -> p k c", c=65)[:, :, 0:64]
                nc.vector.tensor_scalar_mul(
                    dst, tr[:, 0:256].rearrange("p (k c) -> p k c", c=64),
                    cf[:, VSC:VSC + 1])
                bal.charge("dve", 256)
            return f
        bg_tasks.append(mk_vtrans(0, v0))
        bg_tasks.append(mk_vtrans(1, v1))

    oT_tiles = {}

    def emit_div(J):
        """INLINE at chunk-J end: den reciprocal + broadcast + oT divide.
        Reads the pv psum tiles, so must precede the next pv acquisition."""
        oT = otp.tile([128, CK], BF16, tag="oT")
        oT_tiles[J] = oT
        pv0t, pv1t = pv_tiles.pop(J)
        rdB = []
        for hh, pvt in ((0, pv0t), (1, pv1t)):
            den = rdp.tile([1, CK], F32, tag=f"den{hh}")
            nc.vector.tensor_copy(den[:], pvt[64:65, :])
            rd = rdp.tile([1, CK], F32, tag=f"rd{hh}")
            nc.vector.reciprocal_approx_fast(rd[:], den[:])
            bal.charge("dve", 2 * CK)
            rb = rdp.tile([64, CK], F32, tag=f"rdB{hh}")
            nc.gpsimd.partition_broadcast(rb[:], rd[:], channels=64)
            rdB.append(rb)
        for hh, pvt in ((0, pv0t), (1, pv1t)):
            hsl = slice(hh * 64, (hh + 1) * 64)
            nc.vector.tensor_mul(oT[hsl, :], pvt[0:64, :], rdB[hh][:])
            bal.charge("dve", CK)

    def emit_outproj(J):
        """Queue chunk J's Wo matmuls + output DMA (oT(J) long ready by the
        time these drain, so they never block the PE FIFO)."""
        oT = oT_tiles.pop(J)

        def mk_dblk(dt_):
            def f():
                ps = ps_bg.tile([128, CK], F32, tag="bg")
                nc.tensor.matmul(ps[:], cb[:, W_O + dt_ * 128:W_O + (dt_ + 1) * 128],
                                 oT[:], start=True, stop=True)
                po = pop.tile([128, CK], BF16, tag="po")
                copy_psum(po[:], ps[:], CK)
                nc.sync.dma_start(
                    poT[dt_ * 128:(dt_ + 1) * 128, J * CK:(J + 1) * CK],
                    po[:])
            return f
        for dt_ in range(4):
            bg_tasks.append(mk_dblk(dt_))

    # ---------------- main pipeline ----------------
    pv_tiles = {}
    emit_proj(0)
    drain(99)  # chunk 0 projections up front

    for J in range(NCH):
        if J + 1 < NCH:
            emit_proj(J + 1)
        if J >= 1:
            emit_outproj(J - 1)  # behind proj(J+1) tasks: oT(J-1) is ready
        pv0t = ps_pv.tile([65, CK], F32, tag="pv0")
        pv1t = ps_pv.tile([65, CK], F32, tag="pv1")
        pv_tiles[J] = (pv0t, pv1t)
        nkb = KB_PER_CK * (J + 1)
        qt = q_tiles.pop(J)
        pace["rate"] = (len(bg_tasks) + 1) / (2.0 * nkb)

        def emit_pv(kb):
            p = kb - KB_PER_CK * J
            col0 = KBLK * p if p >= 0 else 0
            for hh, vsb, pv in ((0, v0, pv0t), (1, v1, pv1t)):
                nc.tensor.matmul(
                    pv[:, col0:], vsb[:, kb * 65:(kb + 1) * 65],
                    kb_ets[kb][hh][:, col0:],
                    start=(kb == 0), stop=(kb == nkb - 1))
            del kb_ets[kb]

        kb_ets = {}
        for kb in range(nkb):
            p = kb - KB_PER_CK * J
            col0 = KBLK * p if p >= 0 else 0
            w = CK - col0
            sts, ets = [], []
            for hh in range(2):
                st = ps_st.tile([128, CK], F32, tag=f"st{hh}{kb % 2}")
                hsl = slice(hh * 64, (hh + 1) * 64)
                nc.tensor.matmul(
                    st[:, col0:], kT[hsl, kb * KBLK:(kb + 1) * KBLK],
                    qt[hsl, col0:], start=True, stop=True)
                sts.append(st)
            drain_paced()
            for hh in range(2):
                et = etp.tile([128, CK], BF16, tag="et")
                eng = bal.pick(w)
                if eng == "act":
                    nc.scalar.activation(
                        et[:, col0:], sts[hh][:, col0:], EXP,
                        bias=cf[:, BACT + kb:BACT + kb + 1], scale=0.125)
                else:
                    nc.vector.scalar_tensor_tensor(
                        et[:, col0:].bitcast(I16), sts[hh][:, col0:],
                        float(A128),
                        cf[:, KB23 + kb:KB23 + kb + 1].to_broadcast((128, w)),
                        op0=MULT, op1=ADD)
                if p >= 0:
                    nc.vector.tensor_mul(
                        et[:, col0:col0 + KBLK], et[:, col0:col0 + KBLK],
                        cb[:, TRI:TRI + 128])
                    bal.charge("dve", KBLK / 2)
                ets.append(et)
            kb_ets[kb] = ets
            # software pipeline: PV(kb-1) sits BEHIND QK(kb) in the PE FIFO
            # so the PE streams QK(kb) while exp(kb-1) finishes
            if kb >= 1:
                emit_pv(kb - 1)
            drain_paced()
        emit_pv(nkb - 1)
        emit_div(J)
        drain(1)

    emit_outproj(NCH - 1)
    drain(99)


_CACHED = None


def _build():
    global _CACHED
    if _CACHED is not None:
        return _CACHED
    nc = bacc.Bacc("TRN2", target_bir_lowering=False, debug=False,
                   enable_asserts=False, num_devices=NCORES)
    names = [
        ("xT", [D, S], BF16), ("wpack", [128, 2240], BF16),
        ("fpack", [128, 68], F32),
    ]
    aps = [nc.dram_tensor(n, sh, dt_, kind="ExternalInput").ap()
           for n, sh, dt_ in names]
    poT = nc.dram_tensor("poT", [D, S], BF16, kind="ExternalOutput").ap()
    with tile.TileContext(nc) as tc, ExitStack() as ctx:
        _emit(nc, tc, ctx, aps + [poT])
    nc.compile()
    _CACHED = nc
    return nc


def _host_inputs(x, attention_mask, Wq, bq, Wk, bk, Wv, bv, Wo, bo):
    f = np.float32
    x = np.asarray(x, f)
    mask = np.asarray(attention_mask)
    Wq, Wk, Wv, Wo = (np.asarray(w, f) for w in (Wq, Wk, Wv, Wo))
    bq, bk, bv = (np.asarray(b_, f) for b_ in (bq, bk, bv))
    tri = np.triu(np.ones((128, 128), NPBF16))      # [k,q]: 1 where q >= k
    id2 = np.tile(np.eye(64, dtype=NPBF16), (2, 1))
    delta = ((np.arange(128) * PHI) % 1.0).astype(f)          # per key%128
    vscale = (2.0 ** -delta)[:, None].astype(f)
    in_maps = []
    for c in range(NCORES):
        b = c // 4
        h0 = 2 * (c % 4)
        hsl = slice(64 * h0, 64 * h0 + 128)

        def pack_w(W):
            wt = W[hsl, :].T                        # [512, 128] = Wh^T
            return np.ascontiguousarray(
                wt.reshape(4, 128, 128).transpose(1, 0, 2)
                .reshape(128, 512).astype(NPBF16))

        wo_t = Wo[:, hsl].T.astype(NPBF16)           # [128, 512]
        mk = np.where(mask[b] != 0, f(0.0), f(NEG)).astype(f)  # [S]
        mk = mk.reshape(32, 128).T                   # [128 part, 32 kb]
        kb23 = (128.0 * (127.0 + CSH) + 128.0 * delta)[:, None] + \
            np.where(mk < 0, f(-1e9), f(0.0))
        biasact = (delta * np.log(2.0))[:, None] + mk
        wpack = np.concatenate(
            [pack_w(Wq), pack_w(Wk), pack_w(Wv), wo_t, tri, id2], axis=1)
        fpack = np.concatenate(
            [np.stack([bq[hsl], bk[hsl], bv[hsl]], axis=1).astype(f),
             kb23.astype(f), biasact.astype(f), vscale], axis=1)
        in_maps.append({
            "xT": np.ascontiguousarray(x[b].T.astype(NPBF16)),
            "wpack": np.ascontiguousarray(wpack),
            "fpack": np.ascontiguousarray(fpack),
        })
    return in_maps


def _assemble(results, bo):
    out = np.zeros((B, S, D), np.float32)
    for c in range(NCORES):
        out[c // 4] += results[c]["poT"].astype(np.float32).T
    out += np.asarray(bo, np.float32)
    return out


def kernel(**inputs) -> np.ndarray:
    nc = _build()
    in_maps = _host_inputs(**inputs)
    last_err = None
    for attempt in range(3):
        try:
            res = bass_utils.run_bass_kernel_spmd(
                nc, in_maps, core_ids=list(range(NCORES)))
            out = _assemble(res.results, inputs["bo"])
        except Exception as e:  # transient NRT/axon device errors
            last_err = e
            continue
        if np.isfinite(out).all():
            return out
        last_err = RuntimeError("non-finite output")
    raise last_err


def run_traced(inputs, **kwargs):
    """test.py helper: run with NTFF tracing, return (out, BassKernelResults)."""
    nc = _build()
    in_maps = _host_inputs(**inputs)
    res = bass_utils.run_bass_kernel_spmd(
        nc, in_maps, core_ids=list(range(NCORES)), trace=True, **kwargs)
    return _assemble(res.results, inputs["bo"]), res
